# revision 48
# baseline (speedup 1.0000x reference)
"""CoAttention Trainium2 kernel (bf16, host-preprocessed operands).

Problem: B=16, PLEN=1024, QLEN=256, D=256 fp32.
  score[b,p,q] = passage.w_p + question.w_q + (passage*w_pq).question + b
  masked-softmax both ways; returns (p2q_attention, coattention).

Strategy: data-parallel over batch across 8 NeuronCores (2 batches/core).
All heavy lifting is bf16 on the PE; cheap linear preprocessing of the
inputs happens on the host and ships pre-packed:

  host:  PT = P^T, QWT = (Q*w_pq)^T, QG = Q*g, PH = P*h,
         g = exp(Q.w_q + b)*(1-qm), h = exp(P.w_p)*(1-pm), kp = 1-pm
  device per batch (all matmuls bf16, 1 cycle/row):
         S0 = PT^T @ QWT              [p,q]  (pure pq cross term)
         E  = exp(S0) bf16;  ET = E^T via matmul-with-identity-rhs
         aq = sum_p E*PH -> [q,d]; dq = sum_p E*h; dp = sum_q ET*g
         p2q = diag(kp/(dp+eps)) . ET^T @ QG
         q2p = diag(g/(dq+eps))  . aq        (g rides into coatt)
         co  = diag(kp/(dp+eps)) . ET^T @ q2p
  Row-constant exp(sp)/exp(sq) cancel inside each softmax; masks enter as
  exact multiplicative zeros in g/h/kp.

Engine split: PE matmuls/transposes; ACT exps + copies/scales; DVE
copies + reciprocal chains; Pool (gpsimd is SBUF-only -- it cannot touch
PSUM) applies per-row scales on raw bf16 copies where latency allows.
Batch 0 runs a software-pipelined order; batch 1 (the last) front-loads
its S0 burst and aq so the coattention tail drains with direct
ACT/DVE scales and per-pair DMAs.  Outputs are bf16, upcast on the host.

walrus quirks: only ONE sync-wait per non-matmul instruction and none on
matmuls (BIR post-pass splits waits into EventSemaphore carriers); PSUM
accumulation groups must be CONTIGUOUS runs in the PE stream.
"""

import numpy as np
import orjson

import concourse.bass as bass
import concourse.mybir as mybir
import concourse.tile as tile
from concourse.bass_utils import run_bass_kernel_spmd
from concourse.masks import make_identity

F32 = mybir.dt.float32
BF16 = mybir.dt.bfloat16
AF = mybir.ActivationFunctionType

N_CORES = 8
B, PLEN, QLEN, D = 16, 1024, 256, 256
NB = B // N_CORES  # batches per core
PT_T = PLEN // 128  # 8 p-tiles
QT_T = QLEN // 128  # 2 q-tiles
DT_T = D // 128  # 2 d-tiles
EPS = 1e-30

# ---------------------------------------------------------------------------
# walrus single-wait workaround


def _split_waits_in_bir(bir: dict) -> None:
    for f in bir.get("functions", []):
        for blk in f.get("blocks", []):
            out = []
            for i in blk.get("instructions", []):
                si = i.get("sync_info")
                ow = (si or {}).get("on_wait") or []
                limit = 0 if i.get("opcode") == "Matmult" else 1
                if len(ow) > limit:
                    for k, w in enumerate(ow[limit:]):
                        out.append(
                            {
                                "debug": i.get("debug"),
                                "engine": i["engine"],
                                "ins": [],
                                "outs": [],
                                "name": f"{i['name']}__w{k}",
                                "opcode": "EventSemaphore",
                                "sync_info": {"on_update": [], "on_wait": [w]},
                            }
                        )
                    si["on_wait"] = ow[:limit]
                out.append(i)
            blk["instructions"] = out


_patched = False


def _install_bir_wait_split():
    global _patched
    if _patched:
        return
    _patched = True
    import concourse.bass2jax as b2j
    import concourse.bass_utils as bu

    orig = bu.compile_bir_kernel

    def patched(bir_json, tmpdir, neff_name="file.neff"):
        bir = orjson.loads(bir_json)
        _split_waits_in_bir(bir)
        return orig(orjson.dumps(bir), tmpdir, neff_name)

    bu.compile_bir_kernel = patched
    b2j.compile_bir_kernel = patched


# ---------------------------------------------------------------------------


def build_nc() -> bass.Bass:
    nc = bass.Bass()
    pt_d = nc.declare_dram_parameter("pt", [NB, D, PLEN], BF16, isOutput=False)
    qwt_d = nc.declare_dram_parameter("qwt", [NB, D, QLEN], BF16, isOutput=False)
    qg_d = nc.declare_dram_parameter("qg", [NB, QLEN, D], BF16, isOutput=False)
    ph_d = nc.declare_dram_parameter("ph", [NB, PLEN, D], BF16, isOutput=False)
    # aux layouts [128, NB, 20]: cols 0..15 per-p-tile scalars (dup pairs),
    # 16..19 per-q-tile scalars. auxf: kp | g (f32), auxb: h | g (bf16).
    auxf_d = nc.declare_dram_parameter("auxf", [128, NB, 20], F32, isOutput=False)
    auxb_d = nc.declare_dram_parameter("auxb", [128, NB, 20], BF16, isOutput=False)
    out_p2q = nc.declare_dram_parameter("p2q", [NB, PLEN, D], BF16, isOutput=True)
    out_co = nc.declare_dram_parameter("coatt", [NB, PLEN, D], BF16, isOutput=True)

    with tile.TileContext(nc) as tc:
        with (
            tc.tile_pool(name="const", bufs=1) as const_pool,
            tc.tile_pool(name="big", bufs=2) as big,
            tc.tile_pool(name="small", bufs=2) as small,
            tc.tile_pool(name="s0_ps", bufs=2, space="PSUM") as s0_ps,
            tc.tile_pool(name="tp_ps", bufs=2, space="PSUM") as tp_ps,
            tc.tile_pool(name="aq_ps", bufs=1, space="PSUM") as aq_ps,
            tc.tile_pool(name="apco_ps", bufs=2, space="PSUM") as apco_ps,
            tc.tile_pool(name="dn_ps", bufs=1, space="PSUM") as dn_ps,
        ):
            ident = const_pool.tile([128, 128], F32, name="ident")
            make_identity(nc, ident[:])
            identb = const_pool.tile([128, 128], BF16, name="identb")
            nc.vector.tensor_copy(identb[:], ident[:])
            # warm the ACT exp table during the DMA head (first real exp
            # would otherwise pay a ~1.3us table load mid-pipeline)
            warm = const_pool.tile([128, 2], F32, name="warm")
            nc.scalar.activation(warm[:], ident[:, 0:2], AF.Exp)
            auxf = const_pool.tile([128, NB, 20], F32, name="auxf")
            auxb = const_pool.tile([128, NB, 20], BF16, name="auxb")
            # one denominator bank, both batches (disjoint columns):
            # per batch: cols 2t dp (dup pairs, t<8), 16+2qj dq
            denom = dn_ps.tile([128, NB, 20], F32, name="denom")

            # ---- input loads, SP queue order ------------------------------
            loads = []
            for bi in range(NB):
                qwt_sb = big.tile([128, DT_T, QLEN], BF16, name="qwt_sb", tag="qwt")
                pt_sb = big.tile([128, DT_T, PLEN], BF16, name="pt_sb", tag="pt")
                qg_sb = big.tile([128, QT_T, D], BF16, name="qg_sb", tag="qg")
                ph_sb = big.tile([128, PT_T, D], BF16, name="ph_sb", tag="ph")
                qwt_src = qwt_d[bi].rearrange("(j dd) q -> dd j q", dd=128)
                pt_src = pt_d[bi].rearrange("(j dd) p -> dd j p", dd=128)
                qg_src = qg_d[bi].rearrange("(t q) d -> q t d", q=128)
                ph_src = ph_d[bi].rearrange("(t p) d -> p t d", p=128)
                if bi == 0:
                    nc.sync.dma_start(qwt_sb[:], qwt_src)
                    for c0, c1 in ((0, 256), (256, 512), (512, 1024)):
                        nc.sync.dma_start(pt_sb[:, :, c0:c1], pt_src[:, :, c0:c1])
                    nc.sync.dma_start(auxb[:], auxb_d[:])
                    nc.sync.dma_start(auxf[:], auxf_d[:])
                    nc.sync.dma_start(qg_sb[:], qg_src)
                else:
                    # b1's first S0 pair data rides early; rest after b0's ph
                    nc.sync.dma_start(qwt_sb[:], qwt_src)
                    nc.sync.dma_start(pt_sb[:, :, 0:256], pt_src[:, :, 0:256])
                    nc.sync.dma_start(pt_sb[:, :, 256:512], pt_src[:, :, 256:512])
                    b0_ph, b0_ph_src = loads[0][4], loads[0][5]
                    for hf in range(2):
                        nc.sync.dma_start(
                            b0_ph[:, hf * 4 : (hf + 1) * 4, :],
                            b0_ph_src[:, hf * 4 : (hf + 1) * 4, :],
                        )
                    nc.sync.dma_start(pt_sb[:, :, 512:1024], pt_src[:, :, 512:1024])
                    nc.sync.dma_start(qg_sb[:], qg_src)
                    for hf in range(2):
                        nc.sync.dma_start(
                            ph_sb[:, hf * 4 : (hf + 1) * 4, :],
                            ph_src[:, hf * 4 : (hf + 1) * 4, :],
                        )
                loads.append((qwt_sb, pt_sb, qg_sb, ph_sb, ph_sb if bi else ph_sb, ph_src))

            def emit_batch(bi, burst):
                qwt_sb, pt_sb, qg_sb, ph_sb = loads[bi][:4]
                p2q_dst = out_p2q[bi].rearrange("(t p) d -> p t d", p=128)
                co_dst = out_co[bi].rearrange("(t p) d -> p t d", p=128)

                e_sb = big.tile([128, PT_T, QLEN], BF16, name="e_sb", tag="e")
                et_sb = big.tile([128, PT_T, QT_T, 128], BF16, name="et_sb", tag="et")
                p2qr = big.tile([128, PT_T, D], BF16, name="p2qr", tag="p2qr")
                p2q_sb = big.tile([128, PT_T, D], BF16, name="p2q_sb", tag="p2q")
                cor = big.tile([128, PT_T, D], BF16, name="cor", tag="cor")
                co_sb = big.tile([128, PT_T, D], BF16, name="co_sb", tag="co")
                q2p_sb = small.tile([128, QT_T, D], BF16, name="q2p_sb", tag="q2p")
                rp = small.tile([128, 16], F32, name="rp", tag="rp")
                sv = small.tile([128, 4], F32, name="sv", tag="sv")
                uA = small.tile([128, 8], F32, name="uA", tag="uA")
                rA = small.tile([128, 8], F32, name="rA", tag="rA")
                uB = small.tile([128, 8], F32, name="uB", tag="uB")
                rB = small.tile([128, 8], F32, name="rB", tag="rB")
                uS = small.tile([128, 4], F32, name="uS", tag="uS")
                rS = small.tile([128, 4], F32, name="rS", tag="rS")
                dn = denom[:, bi, :]

                def emit_s0(k):
                    s0 = s0_ps.tile([128, 2, QLEN], F32, name="s0", tag="s0")
                    for ti in range(2):
                        t = 2 * k + ti
                        for j in range(DT_T):
                            nc.tensor.matmul(
                                s0[:, ti, :],
                                pt_sb[:, j, t * 128 : (t + 1) * 128],
                                qwt_sb[:, j, :],
                                start=(j == 0),
                                stop=(j == DT_T - 1),
                            )
                    nc.scalar.activation(e_sb[:, 2 * k : 2 * k + 2, :], s0[:], AF.Exp)

                def emit_t(m, dve_copy):
                    tp = tp_ps.tile([128, 2, QT_T, 128], F32, name="tp", tag="tp")
                    for ti in range(2):
                        t = 2 * m + ti
                        for qj in range(QT_T):
                            nc.tensor.matmul(
                                tp[:, ti, qj, :],
                                e_sb[:, t, qj * 128 : (qj + 1) * 128],
                                identb[:],
                                start=True,
                                stop=True,
                            )
                    if dve_copy:
                        nc.vector.tensor_copy(et_sb[:, 2 * m : 2 * m + 2, :, :], tp[:])
                    else:
                        nc.scalar.copy(et_sb[:, 2 * m : 2 * m + 2, :, :], tp[:])

                def emit_t_dma(m):
                    # ET via the DMA crossbar: blocks arrive (t, qj)-major
                    nc.sync.dma_start_transpose(
                        et_sb[:, 2 * m : 2 * m + 2, :, :],
                        e_sb[:, 2 * m : 2 * m + 2, :],
                    )

                def emit_dp(m):
                    for ti in range(2):
                        t = 2 * m + ti
                        for qj in range(QT_T):
                            nc.tensor.matmul(
                                dn[:, 2 * t : 2 * t + 2],
                                et_sb[:, t, qj, :],
                                auxb[:, bi, 16 + 2 * qj : 18 + 2 * qj],
                                start=(qj == 0),
                                stop=(qj == QT_T - 1),
                            )

                def emit_ap(m, dve_copy):
                    ap = apco_ps.tile([128, 2, D], F32, name="ap", tag="apco")
                    for ti in range(2):
                        t = 2 * m + ti
                        for qj in range(QT_T):
                            nc.tensor.matmul(
                                ap[:, ti, :],
                                et_sb[:, t, qj, :],
                                qg_sb[:, qj, :],
                                start=(qj == 0),
                                stop=(qj == QT_T - 1),
                            )
                    if dve_copy:
                        nc.vector.tensor_copy(p2qr[:, 2 * m : 2 * m + 2, :], ap[:])
                    else:
                        nc.scalar.copy(p2qr[:, 2 * m : 2 * m + 2, :], ap[:])

                def emit_aq():
                    a = aq_ps.tile([128, QT_T, D], F32, name="aq", tag="aq")
                    for qj in range(QT_T):
                        for t in range(PT_T):
                            nc.tensor.matmul(
                                a[:, qj, :],
                                e_sb[:, t, qj * 128 : (qj + 1) * 128],
                                ph_sb[:, t, :],
                                start=(t == 0),
                                stop=(t == PT_T - 1),
                            )
                    return a

                def emit_dq():
                    for qj in range(QT_T):
                        for t in range(PT_T):
                            nc.tensor.matmul(
                                dn[:, 16 + 2 * qj : 18 + 2 * qj],
                                e_sb[:, t, qj * 128 : (qj + 1) * 128],
                                auxb[:, bi, 2 * t : 2 * t + 2],
                                start=(t == 0),
                                stop=(t == PT_T - 1),
                            )

                def chain_a():
                    nc.vector.tensor_scalar_add(uA[:], dn[:, 0:8], EPS)
                    nc.vector.reciprocal(rA[:], uA[:])
                    nc.vector.tensor_mul(rp[:, 0:8], rA[:], auxf[:, bi, 0:8])

                def chain_b():
                    nc.vector.tensor_scalar_add(uB[:], dn[:, 8:16], EPS)
                    nc.vector.reciprocal(rB[:], uB[:])
                    nc.vector.tensor_mul(rp[:, 8:16], rB[:], auxf[:, bi, 8:16])

                def chain_s():
                    nc.vector.tensor_scalar_add(uS[:], dn[:, 16:20], EPS)
                    nc.vector.reciprocal(rS[:], uS[:])
                    nc.vector.tensor_mul(sv[:], rS[:], auxf[:, bi, 16:20])

                def q2p_scales(aq):
                    for qj in range(QT_T):
                        nc.scalar.activation(
                            q2p_sb[:, qj, :],
                            aq[:, qj, :],
                            AF.Copy,
                            scale=sv[:, 2 * qj : 2 * qj + 1],
                        )

                def emit_p2q_out(half):
                    for t in range(half * 4, half * 4 + 4):
                        nc.gpsimd.tensor_scalar_mul(
                            p2q_sb[:, t, :], p2qr[:, t, :], rp[:, 2 * t : 2 * t + 1]
                        )
                    nc.sync.dma_start(
                        p2q_dst[:, half * 4 : half * 4 + 4, :],
                        p2q_sb[:, half * 4 : half * 4 + 4, :],
                    )

                def emit_co_pooled(m, dve_copy):
                    copair = apco_ps.tile([128, 2, D], F32, name="copair", tag="apco")
                    for ti in range(2):
                        t = 2 * m + ti
                        for qj in range(QT_T):
                            nc.tensor.matmul(
                                copair[:, ti, :],
                                et_sb[:, t, qj, :],
                                q2p_sb[:, qj, :],
                                start=(qj == 0),
                                stop=(qj == QT_T - 1),
                            )
                    if dve_copy:
                        nc.vector.tensor_copy(cor[:, 2 * m : 2 * m + 2, :], copair[:])
                    else:
                        nc.scalar.copy(cor[:, 2 * m : 2 * m + 2, :], copair[:])
                    for ti in range(2):
                        t = 2 * m + ti
                        nc.gpsimd.tensor_scalar_mul(
                            co_sb[:, t, :], cor[:, t, :], rp[:, 2 * t : 2 * t + 1]
                        )
                    if m % 2 == 1:
                        nc.sync.dma_start(
                            co_dst[:, 2 * m - 2 : 2 * m + 2, :],
                            co_sb[:, 2 * m - 2 : 2 * m + 2, :],
                        )

                def emit_co_direct(m, pool=None, tile_dma=False, dma=None):
                    # direct PSUM->SBUF scales, one on ACT one on DVE
                    # (parallel); the S0 pool banks are free by now
                    pool = pool or s0_ps
                    tag = "s0" if pool is s0_ps else "apco"
                    copair = pool.tile([128, 2, D], F32, name="copair", tag=tag)
                    for ti in range(2):
                        t = 2 * m + ti
                        for qj in range(QT_T):
                            nc.tensor.matmul(
                                copair[:, ti, :],
                                et_sb[:, t, qj, :],
                                q2p_sb[:, qj, :],
                                start=(qj == 0),
                                stop=(qj == QT_T - 1),
                            )
                    t0, t1 = 2 * m, 2 * m + 1
                    nc.vector.tensor_scalar_mul(
                        co_sb[:, t0, :], copair[:, 0, :], rp[:, 2 * t0 : 2 * t0 + 1]
                    )
                    nc.scalar.activation(
                        co_sb[:, t1, :],
                        copair[:, 1, :],
                        AF.Copy,
                        scale=rp[:, 2 * t1 : 2 * t1 + 1],
                    )
                    if dma == "tiles":
                        nc.sync.dma_start(co_dst[:, t0 : t0 + 1, :], co_sb[:, t0 : t0 + 1, :])
                        nc.sync.dma_start(co_dst[:, t1 : t1 + 1, :], co_sb[:, t1 : t1 + 1, :])
                    elif dma is False:
                        pass
                    elif isinstance(dma, tuple):
                        nc.sync.dma_start(
                            co_dst[:, dma[0] : dma[1], :], co_sb[:, dma[0] : dma[1], :]
                        )
                    else:
                        nc.sync.dma_start(
                            co_dst[:, 2 * m : 2 * m + 2, :], co_sb[:, 2 * m : 2 * m + 2, :]
                        )

                if not burst:
                    # -------- software-pipelined (first batch) --------
                    emit_s0(0)
                    emit_s0(1)
                    emit_t(0, dve_copy=False)
                    emit_dp(0)
                    emit_s0(2)
                    emit_t(1, dve_copy=True)
                    emit_dp(1)
                    emit_ap(0, dve_copy=False)
                    emit_s0(3)
                    emit_t(2, dve_copy=False)
                    emit_dp(2)
                    emit_ap(1, dve_copy=True)
                    chain_a()
                    emit_p2q_out(0)
                    emit_t(3, dve_copy=True)
                    emit_dp(3)
                    chain_b()
                    emit_dq()
                    chain_s()
                    aq = emit_aq()
                    q2p_scales(aq)
                    emit_ap(2, dve_copy=False)
                    emit_ap(3, dve_copy=True)
                    emit_p2q_out(1)
                    yield
                    emit_co_pooled(0, dve_copy=False)
                    emit_co_pooled(1, dve_copy=True)
                    emit_co_pooled(2, dve_copy=False)
                    emit_co_pooled(3, dve_copy=True)
                    yield
                else:
                    # -------- burst (last batch): S0s and aq early, short
                    # co drain with direct scales and per-pair DMAs --------
                    emit_s0(0)
                    emit_t_dma(0)
                    emit_s0(1)
                    emit_t_dma(1)
                    emit_s0(2)
                    emit_t_dma(2)
                    emit_s0(3)
                    emit_t_dma(3)
                    yield
                    emit_dq()
                    chain_s()
                    aq = emit_aq()
                    q2p_scales(aq)
                    emit_dp(0)
                    emit_dp(1)
                    chain_a()
                    emit_ap(0, dve_copy=False)
                    emit_ap(1, dve_copy=True)
                    emit_p2q_out(0)
                    emit_ap(2, dve_copy=False)
                    emit_ap(3, dve_copy=True)
                    emit_dp(2)
                    emit_dp(3)
                    chain_b()
                    emit_p2q_out(1)
                    emit_co_pooled(0, dve_copy=False)
                    emit_co_pooled(1, dve_copy=True)
                    emit_co_direct(2)
                    emit_co_direct(3, pool=apco_ps)
                    yield

            # interleave: b0 body | b1 first S0s | b0 co tail | b1 rest
            g0 = emit_batch(0, burst=False)
            g1 = emit_batch(1, burst=True)
            next(g0)  # b0 body
            next(g1)  # b1 s0 burst
            next(g0, None)  # b0 co tail
            next(g1, None)  # b1 rest

    return nc


_nc_cache = None


def _preprocess(passage, question, passage_mask, question_mask, W, b):
    import ml_dtypes

    BF = ml_dtypes.bfloat16
    p = np.ascontiguousarray(passage, dtype=np.float32)
    q = np.ascontiguousarray(question, dtype=np.float32)
    W = np.asarray(W, dtype=np.float32)
    b = np.asarray(b, dtype=np.float32)
    pm = np.asarray(passage_mask, dtype=np.float32)
    qm = np.asarray(question_mask, dtype=np.float32)
    d = D
    w_p, w_q, w_pq = W[:d], W[d : 2 * d], W[2 * d :]
    sp = p @ w_p  # [B, PLEN]
    sq = q @ w_q  # [B, QLEN]
    g = np.exp(sq + b[0]) * (1.0 - qm)  # [B, QLEN]
    h = np.exp(sp) * (1.0 - pm)  # [B, PLEN]
    kp = 1.0 - pm  # [B, PLEN]

    pt = np.ascontiguousarray(p.transpose(0, 2, 1)).astype(BF)
    qwt = np.ascontiguousarray((q * w_pq[None, None, :]).transpose(0, 2, 1)).astype(BF)
    qg = np.ascontiguousarray(q * g[:, :, None]).astype(BF)
    ph = np.ascontiguousarray(p * h[:, :, None]).astype(BF)

    def tile_cols(x, nt):  # [B, nt*128] -> [128, B, 2*nt] dup pairs
        y = x.reshape(B, nt, 128).transpose(2, 0, 1)  # [128, B, nt]
        return np.repeat(y, 2, axis=2)

    auxf = np.concatenate([tile_cols(kp, 8), tile_cols(g, 2)], axis=2).astype(
        np.float32
    )  # [128, B, 20]
    auxb = np.concatenate([tile_cols(h, 8), tile_cols(g, 2)], axis=2).astype(BF)
    return pt, qwt, qg, ph, auxf, auxb


def kernel(passage, question, passage_mask, question_mask, W, b):
    global _nc_cache
    _install_bir_wait_split()
    if _nc_cache is None:
        _nc_cache = build_nc()
    nc = _nc_cache

    pt, qwt, qg, ph, auxf, auxb = _preprocess(
        passage, question, passage_mask, question_mask, W, b
    )

    in_maps = []
    for c in range(N_CORES):
        s = slice(c * NB, (c + 1) * NB)
        in_maps.append(
            {
                "pt": np.ascontiguousarray(pt[s]),
                "qwt": np.ascontiguousarray(qwt[s]),
                "qg": np.ascontiguousarray(qg[s]),
                "ph": np.ascontiguousarray(ph[s]),
                "auxf": np.ascontiguousarray(auxf[:, s]),
                "auxb": np.ascontiguousarray(auxb[:, s]),
            }
        )
    res = run_bass_kernel_spmd(nc, in_maps, list(range(N_CORES)))
    p2q = np.concatenate(
        [np.asarray(r["p2q"], dtype=np.float32) for r in res.results], axis=0
    )
    coatt = np.concatenate(
        [np.asarray(r["coatt"], dtype=np.float32) for r in res.results], axis=0
    )
    return p2q, coatt
